# revision 30
# baseline (speedup 1.0000x reference)
"""Trainium2 Bass kernel for nn_DecoderTF (masked spectrogram decode + overlap-add).

Computation (per batch m, channel c):
    masked[n, k] = inputs[m, n, k] * est_mask[m, c, n, k]          n in [0,512), k in [0,6000)
    frames[k, l] = sum_n masked[n, k] * W[n, l]                    l in [0,16)
    out[m, c, t] = overlap_and_add(frames, hop=8)                  t in [0,48008)

With L=16 and hop=8, overlap-add reduces to a two-term sum; viewing the output
as out2d[6001, 8]:
    out2d[k, j] = frames[k, j] + frames[k-1, j+8]
Each masked column is streamed through the PE ONCE against a 40-wide stationary
holding W[:, 0:8] at cols 0..7 and W[:, 8:16] at cols 32..39 (zeros between, so
the B half lands at PSUM partition 32 -- engine partition offsets must be
quadrant-aligned).  Both OLA halves come out of the same pass; ACT copies the B
half to SBUF (engines may read only one PSUM operand) and a DVE shifted add
merges:
    fr[j, k] = po[j, k] + cb[j, k-1]
The k-1 column for a chunk's first k lives in the previous chunk's cb SBUF tile
(still alive in the round-robin pool), so no halo columns, no memsets, and the
PE streams half the columns of the two-pass formulation.

Sharding: data-parallel over M — core m handles inputs[m] / est_mask[m] (no
cross-core communication, W replicated).  Per-core HBM traffic ~37 MB at
~385 GB/s is the ~97 us roofline for this kernel.
"""

import sys

for _p in ("/opt/trn_rl_repo",):
    if _p not in sys.path:
        sys.path.insert(0, _p)

import numpy as np

import concourse.bass as bass
import concourse.mybir as mybir
from concourse import bacc, masks
from concourse.tile import TileContext
from concourse.bass_utils import run_bass_kernel_spmd

N, L, HOP = 512, 16, 8
K = 6000
C = 2
M = 8
T_OUT = (K - 1) * HOP + L  # 48008
R = K + 1                  # rows of out2d: out2d[k, j] = out[k*8 + j]

F32 = mybir.dt.float32
# float32r streams fp32 bits through the PE's single-pass (reduced internal
# precision) path: 1 cycle/row vs 4 for full fp32.
MM_DT = mybir.dt.float32r

# k-slices (over out2d rows / masked columns), each a single <=512-wide PSUM
# chunk.  Monotonic in k (the shifted add reads the previous chunk's B half);
# uniform 512 keeps the DMA-completion granularity fine so DVE never waits
# long for data; narrow last slices for a short drain tail.
KSLICES = [(q, 512) for q in range(0, 5632, 512)] + [
    (5632, 256),
    (5888, R - 5888),  # 113 wide
]
KSLICES_MIXED = [
    (0, 512),
    (512, 512),
    (1024, 1024),
    (2048, 1024),
    (3072, 1024),
    (4096, 1024),
    (5120, 512),
    (5632, 256),
    (5888, R - 5888),
]


def _build_nc(kslices=None, bufs=None, m_ring=None, lag=0):
    kslices = kslices or KSLICES
    maxw = max(wks for _, wks in kslices)
    if bufs is None:
        # defaults scaled so SBUF stays ~180KB/partition
        bufs = {"x": 3, "m": 4, "mk": 12} if maxw > 512 else {}
    mxs = 8 * ((maxw + 127) // 128)  # out-tile cols for the widest slice
    nc = bacc.Bacc()
    x = nc.declare_dram_parameter("x", [N, K], F32, isOutput=False)
    mk = nc.declare_dram_parameter("mask", [C, N, K], F32, isOutput=False)
    w = nc.declare_dram_parameter("w", [N, L], MM_DT, isOutput=False)
    out = nc.declare_dram_parameter("out", [C, T_OUT], F32, isOutput=True)

    with TileContext(nc) as tc:
        with (
            tc.tile_pool(name="wp", bufs=1) as wp,
            tc.tile_pool(name="idp", bufs=1) as idp,
            tc.tile_pool(name="xp", bufs=bufs.get("x", 6)) as xp,
            tc.tile_pool(name="mp", bufs=bufs.get("m", 10)) as mp,
            tc.tile_pool(name="mkp", bufs=bufs.get("mk", 20)) as mkp,
            tc.tile_pool(name="frp", bufs=3) as frp,
            tc.tile_pool(name="cbp", bufs=4) as cbp,
            tc.tile_pool(name="outp", bufs=4) as outp,
            tc.tile_pool(name="pop", bufs=bufs.get("po", 7), space="PSUM") as pop,
            tc.tile_pool(name="ptp", bufs=bufs.get("pt", 1), space="PSUM") as ptp,
        ):
            # W, stationary slices 40 cols: [p, 40n + j] = W[128n + p, j] for
            # j<8, [p, 40n + 32 + j] = W[128n + p, 8 + j], zeros between.
            SW = 40
            w_t = wp.tile([128, 4 * SW], MM_DT)
            nc.vector.memset(w_t[:, :].bitcast(mybir.dt.uint32), 0)
            id_t = idp.tile([8, 8], F32)
            masks.make_identity(nc, id_t[:, :])
            m_eng = nc.scalar if m_ring == "scalar" else nc.sync

            def load_w():
                # issued after the first input DMAs so the big stream ramps
                # immediately; W is only needed by the first matmul ~10us in
                nc.sync.dma_start(
                    out=w_t[:, :].rearrange("p (n q) -> p n q", n=4)[
                        :, :, 0:8
                    ],
                    in_=w.rearrange("(n p) l -> p n l", p=128)[:, :, 0:8],
                )
                nc.sync.dma_start(
                    out=w_t[:, :].rearrange("p (n q) -> p n q", n=4)[
                        :, :, 32:40
                    ],
                    in_=w.rearrange("(n p) l -> p n l", p=128)[:, :, 8:16],
                )

            # per-channel previous chunk's B-half SBUF copy: (tile, wa)
            prev_cb = {}

            def emit_tail(o0, wks, pos):
                """Post-PSUM work for one k-slice: merge OLA halves, transpose,
                stream out.  Emitted one slice late so DVE's in-order stream
                never makes the next slice's multiplies wait on PE results."""
                for c in range(C):
                    fr_t = frp.tile([8, maxw], F32, tag="fr")
                    for q0, wch, wa, po in pos[c]:
                        fo = q0 - o0
                        # B half to SBUF shifted one col right (cb[i] = B col
                        # q0+i-1); col 0 = previous chunk's last B col, copied
                        # by ACT so DVE does a single add per chunk.
                        cb = cbp.tile([8, 513], F32, tag="cb")
                        nc.scalar.copy(cb[:, 1 : 1 + wa], po[32:40, 0:wa])
                        if q0 == 0:
                            nc.vector.tensor_add(
                                fr_t[:, 1:wa], po[0:8, 1:wa], cb[:, 1:wa]
                            )
                            nc.vector.tensor_copy(fr_t[:, 0:1], po[0:8, 0:1])
                        else:
                            pcb, pwa = prev_cb[c]
                            nc.scalar.copy(cb[:, 0:1], pcb[:, pwa : pwa + 1])
                            nc.vector.tensor_add(
                                fr_t[:, fo : fo + wa],
                                po[0:8, 0:wa],
                                cb[:, 0:wa],
                            )
                        if q0 + wch > K:
                            # final out2d row k=K: B-term only
                            nc.scalar.copy(
                                fr_t[:, fo + wa : fo + wa + 1],
                                cb[:, wa : wa + 1],
                            )
                        prev_cb[c] = (cb, wa)

                    # transpose [8, 128] slabs -> [128, 8] into one PSUM bank
                    n_sub = (wks + 127) // 128
                    pt = ptp.tile([128, mxs], F32, tag="pt")
                    for s in range(n_sub):
                        s0 = 128 * s
                        sw = min(128, wks - s0)
                        nc.tensor.transpose(
                            pt[0:sw, 8 * s : 8 * s + 8],
                            fr_t[:, s0 : s0 + sw],
                            id_t[:, :],
                        )
                    ot = outp.tile([128, mxs], F32, tag="ot")
                    nc.scalar.copy(ot[:, 0 : 8 * n_sub], pt[:, 0 : 8 * n_sub])
                    # stream this slice's output rows out now, on the ACT HWDGE
                    # ring so the many tiny strided descriptors don't stall the
                    # input stream's (sync) ring.  The last slices' output goes
                    # per-slab so the final descriptor drain is short.
                    s_full = n_sub if wks % 128 == 0 else n_sub - 1
                    t0 = 8 * o0
                    if s_full:
                        per_slab = o0 + wks >= K - 512
                        step = 1 if per_slab else s_full
                        for sb in range(0, s_full, step):
                            v = out[
                                c,
                                t0 + 1024 * sb : t0 + 1024 * (sb + step),
                            ].rearrange("(s p j) -> p s j", p=128, j=8)
                            sv = ot[:, 8 * sb : 8 * (sb + step)].rearrange(
                                "p (s j) -> p s j", j=8
                            )
                            nc.scalar.dma_start(out=v, in_=sv)
                    if s_full != n_sub:
                        rem = wks - 128 * s_full  # 113
                        v2 = out[c, t0 + 1024 * s_full : T_OUT].rearrange(
                            "(p j) -> p j", j=8
                        )
                        nc.scalar.dma_start(
                            out=v2, in_=ot[0:rem, 8 * s_full : 8 * n_sub]
                        )

            pending = None
            for o0, wks in kslices:
                hi = min(o0 + wks, K)
                dlen = hi - o0
                # one big x DMA + one m DMA per channel: 3 ring slots per
                # slice instead of 8, so DMA issue runs ~2.7 slices ahead of
                # the 8-deep per-ring in-flight window.
                x_t = xp.tile([128, 4 * maxw], F32, tag="x")
                nc.sync.dma_start(
                    out=x_t[:, :].rearrange("p (n k) -> p n k", n=4)[
                        :, :, 0:dlen
                    ],
                    in_=x.rearrange("(n p) k -> p n k", p=128)[:, :, o0:hi],
                )
                m_tc = []
                for c in range(C):
                    m_t = mp.tile([128, 4 * maxw], F32, tag="m")
                    m_eng.dma_start(
                        out=m_t[:, :].rearrange("p (n k) -> p n k", n=4)[
                            :, :, 0:dlen
                        ],
                        in_=mk[c].rearrange("(n p) k -> p n k", p=128)[
                            :, :, o0:hi
                        ],
                    )
                    m_tc.append(m_t)
                if o0 == kslices[0][0]:
                    load_w()
                mk_ts = {}
                for n in range(4):
                    for c in range(C):
                        mk_t = mkp.tile([128, maxw], MM_DT, tag="mk")
                        nc.vector.tensor_mul(
                            mk_t[:, 0:dlen],
                            x_t[:, n * maxw : n * maxw + dlen],
                            m_tc[c][:, n * maxw : n * maxw + dlen],
                        )
                        mk_ts[c, n] = mk_t

                # matmuls now (consume mk tiles promptly); tail one slice late
                pos = {c: [] for c in range(C)}
                for c in range(C):
                    q0 = o0
                    while q0 < o0 + wks:
                        wch = min(512, o0 + wks - q0)
                        wa = min(wch, K - q0)  # columns actually streamed
                        po = pop.tile([40, 512], F32, tag="po")
                        for n in range(4):
                            nc.tensor.matmul(
                                po[0:40, 0:wa],
                                w_t[:, SW * n : SW * (n + 1)],
                                mk_ts[c, n][:, q0 - o0 : q0 - o0 + wa],
                                start=(n == 0),
                                stop=(n == 3),
                            )
                        pos[c].append((q0, wch, wa, po))
                        q0 += wch

                if not lag:
                    emit_tail(o0, wks, pos)
                else:
                    if pending is not None:
                        emit_tail(*pending)
                    pending = (o0, wks, pos)
            if pending is not None:
                emit_tail(*pending)
    nc.finalize()
    return nc


_NC_CACHE = None


def _get_nc():
    global _NC_CACHE
    if _NC_CACHE is None:
        import os

        cfg = os.environ.get("KCONF", "")
        kw = {}
        if cfg == "mixed":
            kw = {"kslices": KSLICES_MIXED}
        elif cfg == "mixed_lag":
            kw = {"kslices": KSLICES_MIXED, "lag": 1}
        elif cfg == "lag":
            kw = {"lag": 1}
        elif cfg == "mscalar":
            kw = {"m_ring": "scalar"}
        _NC_CACHE = _build_nc(**kw)
    return _NC_CACHE


def run(inputs, est_mask, W, trace=False):
    """Returns (out [M, C, T_OUT] float32, exec_time_ns or None)."""
    inputs = np.ascontiguousarray(np.asarray(inputs, dtype=np.float32))
    est_mask = np.ascontiguousarray(np.asarray(est_mask, dtype=np.float32))
    W = np.ascontiguousarray(np.asarray(W, dtype=np.float32))
    assert inputs.shape == (M, N, K)
    assert est_mask.shape == (M, C, N, K)
    assert W.shape == (N, L)

    nc = _get_nc()
    in_maps = [
        {"x": inputs[m], "mask": est_mask[m], "w": W} for m in range(M)
    ]
    res = run_bass_kernel_spmd(nc, in_maps, list(range(M)), trace=trace)
    out = np.stack([res.results[m]["out"] for m in range(M)], axis=0)
    return out.astype(np.float32, copy=False), res.exec_time_ns


def kernel(inputs, est_mask, W):
    out, _ = run(inputs, est_mask, W)
    return out


# revision 31
# speedup vs baseline: 1.0370x; 1.0370x over previous
"""Trainium2 Bass kernel for nn_DecoderTF (masked spectrogram decode + overlap-add).

Computation (per batch m, channel c):
    masked[n, k] = inputs[m, n, k] * est_mask[m, c, n, k]          n in [0,512), k in [0,6000)
    frames[k, l] = sum_n masked[n, k] * W[n, l]                    l in [0,16)
    out[m, c, t] = overlap_and_add(frames, hop=8)                  t in [0,48008)

With L=16 and hop=8, overlap-add reduces to a two-term sum; viewing the output
as out2d[6001, 8]:
    out2d[k, j] = frames[k, j] + frames[k-1, j+8]
Each masked column is streamed through the PE ONCE against a 40-wide stationary
holding W[:, 0:8] at cols 0..7 and W[:, 8:16] at cols 32..39 (zeros between, so
the B half lands at PSUM partition 32 -- engine partition offsets must be
quadrant-aligned).  Both OLA halves come out of the same pass; ACT copies the B
half to SBUF (engines may read only one PSUM operand) and a DVE shifted add
merges:
    fr[j, k] = po[j, k] + cb[j, k-1]
The k-1 column for a chunk's first k lives in the previous chunk's cb SBUF tile
(still alive in the round-robin pool), so no halo columns, no memsets, and the
PE streams half the columns of the two-pass formulation.

Sharding: data-parallel over M — core m handles inputs[m] / est_mask[m] (no
cross-core communication, W replicated).  Per-core HBM traffic ~37 MB at
~385 GB/s is the ~97 us roofline for this kernel.
"""

import sys

for _p in ("/opt/trn_rl_repo",):
    if _p not in sys.path:
        sys.path.insert(0, _p)

import numpy as np

import concourse.bass as bass
import concourse.mybir as mybir
from concourse import bacc, masks
from concourse.tile import TileContext
from concourse.bass_utils import run_bass_kernel_spmd

N, L, HOP = 512, 16, 8
K = 6000
C = 2
M = 8
T_OUT = (K - 1) * HOP + L  # 48008
R = K + 1                  # rows of out2d: out2d[k, j] = out[k*8 + j]

F32 = mybir.dt.float32
# float32r streams fp32 bits through the PE's single-pass (reduced internal
# precision) path: 1 cycle/row vs 4 for full fp32.
MM_DT = mybir.dt.float32r

# k-slices (over out2d rows / masked columns), each a single <=512-wide PSUM
# chunk.  Monotonic in k (the shifted add reads the previous chunk's B half);
# uniform 512 keeps the DMA-completion granularity fine so DVE never waits
# long for data; narrow last slices for a short drain tail.
KSLICES = [(q, 512) for q in range(0, 5632, 512)] + [
    (5632, 256),
    (5888, R - 5888),  # 113 wide
]
KSLICES_MIXED = [
    (0, 512),
    (512, 512),
    (1024, 1024),
    (2048, 1024),
    (3072, 1024),
    (4096, 1024),
    (5120, 512),
    (5632, 256),
    (5888, R - 5888),
]


def _build_nc(kslices=None, bufs=None, m_ring=None, lag=0):
    kslices = kslices or KSLICES
    maxw = max(wks for _, wks in kslices)
    if bufs is None:
        # defaults scaled so SBUF stays ~180KB/partition
        bufs = {"x": 3, "m": 4, "mk": 12} if maxw > 512 else {}
    mxs = 8 * ((maxw + 127) // 128)  # out-tile cols for the widest slice
    nc = bacc.Bacc()
    x = nc.declare_dram_parameter("x", [N, K], F32, isOutput=False)
    mk = nc.declare_dram_parameter("mask", [C, N, K], F32, isOutput=False)
    w = nc.declare_dram_parameter("w", [N, L], MM_DT, isOutput=False)
    out = nc.declare_dram_parameter("out", [C, T_OUT], F32, isOutput=True)

    with TileContext(nc) as tc:
        with (
            tc.tile_pool(name="wp", bufs=1) as wp,
            tc.tile_pool(name="idp", bufs=1) as idp,
            tc.tile_pool(name="xp", bufs=bufs.get("x", 6)) as xp,
            tc.tile_pool(name="mp", bufs=bufs.get("m", 10)) as mp,
            tc.tile_pool(name="mkp", bufs=bufs.get("mk", 20)) as mkp,
            tc.tile_pool(name="frp", bufs=3) as frp,
            tc.tile_pool(name="cbp", bufs=4) as cbp,
            tc.tile_pool(name="outp", bufs=4) as outp,
            tc.tile_pool(name="pop", bufs=bufs.get("po", 7), space="PSUM") as pop,
            tc.tile_pool(name="ptp", bufs=bufs.get("pt", 1), space="PSUM") as ptp,
        ):
            # W, stationary slices 40 cols: [p, 40n + j] = W[128n + p, j] for
            # j<8, [p, 40n + 32 + j] = W[128n + p, 8 + j], zeros between.
            SW = 40
            w_t = wp.tile([128, 4 * SW], MM_DT)
            nc.vector.memset(w_t[:, :].bitcast(mybir.dt.uint32), 0)
            id_t = idp.tile([8, 8], F32)
            masks.make_identity(nc, id_t[:, :])
            m_eng = nc.scalar if m_ring == "scalar" else nc.sync

            def load_w():
                # issued after the first input DMAs so the big stream ramps
                # immediately; W is only needed by the first matmul ~10us in
                nc.sync.dma_start(
                    out=w_t[:, :].rearrange("p (n q) -> p n q", n=4)[
                        :, :, 0:8
                    ],
                    in_=w.rearrange("(n p) l -> p n l", p=128)[:, :, 0:8],
                )
                nc.sync.dma_start(
                    out=w_t[:, :].rearrange("p (n q) -> p n q", n=4)[
                        :, :, 32:40
                    ],
                    in_=w.rearrange("(n p) l -> p n l", p=128)[:, :, 8:16],
                )

            # per-channel previous chunk's B-half SBUF copy: (tile, wa)
            prev_cb = {}

            def emit_tail(o0, wks, pos):
                """Post-PSUM work for one k-slice: merge OLA halves, transpose,
                stream out.  Emitted one slice late so DVE's in-order stream
                never makes the next slice's multiplies wait on PE results."""
                for c in range(C):
                    fr_t = frp.tile([8, maxw], F32, tag="fr")
                    for q0, wch, wa, po in pos[c]:
                        fo = q0 - o0
                        # B half to SBUF shifted one col right (cb[i] = B col
                        # q0+i-1); col 0 = previous chunk's last B col, copied
                        # by ACT so DVE does a single add per chunk.
                        cb = cbp.tile([8, 513], F32, tag="cb")
                        nc.scalar.copy(cb[:, 1 : 1 + wa], po[32:40, 0:wa])
                        if q0 == 0:
                            nc.vector.tensor_add(
                                fr_t[:, 1:wa], po[0:8, 1:wa], cb[:, 1:wa]
                            )
                            nc.vector.tensor_copy(fr_t[:, 0:1], po[0:8, 0:1])
                        else:
                            pcb, pwa = prev_cb[c]
                            nc.scalar.copy(cb[:, 0:1], pcb[:, pwa : pwa + 1])
                            nc.vector.tensor_add(
                                fr_t[:, fo : fo + wa],
                                po[0:8, 0:wa],
                                cb[:, 0:wa],
                            )
                        if q0 + wch > K:
                            # final out2d row k=K: B-term only
                            nc.scalar.copy(
                                fr_t[:, fo + wa : fo + wa + 1],
                                cb[:, wa : wa + 1],
                            )
                        prev_cb[c] = (cb, wa)

                    # transpose [8, 128] slabs -> [128, 8] into one PSUM bank
                    n_sub = (wks + 127) // 128
                    pt = ptp.tile([128, mxs], F32, tag="pt")
                    for s in range(n_sub):
                        s0 = 128 * s
                        sw = min(128, wks - s0)
                        nc.tensor.transpose(
                            pt[0:sw, 8 * s : 8 * s + 8],
                            fr_t[:, s0 : s0 + sw],
                            id_t[:, :],
                        )
                    ot = outp.tile([128, mxs], F32, tag="ot")
                    nc.scalar.copy(ot[:, 0 : 8 * n_sub], pt[:, 0 : 8 * n_sub])
                    # stream this slice's output rows out now, on the ACT HWDGE
                    # ring so the many tiny strided descriptors don't stall the
                    # input stream's (sync) ring.  The last slices' output goes
                    # per-slab so the final descriptor drain is short.
                    s_full = n_sub if wks % 128 == 0 else n_sub - 1
                    t0 = 8 * o0
                    if s_full:
                        import os as _os

                        per_slab = (
                            o0 + wks >= K - 512
                            and _os.environ.get("NOSLAB", "") != "1"
                        )
                        step = 1 if per_slab else s_full
                        for sb in range(0, s_full, step):
                            v = out[
                                c,
                                t0 + 1024 * sb : t0 + 1024 * (sb + step),
                            ].rearrange("(s p j) -> p s j", p=128, j=8)
                            sv = ot[:, 8 * sb : 8 * (sb + step)].rearrange(
                                "p (s j) -> p s j", j=8
                            )
                            nc.scalar.dma_start(out=v, in_=sv)
                    if s_full != n_sub:
                        rem = wks - 128 * s_full  # 113
                        v2 = out[c, t0 + 1024 * s_full : T_OUT].rearrange(
                            "(p j) -> p j", j=8
                        )
                        nc.scalar.dma_start(
                            out=v2, in_=ot[0:rem, 8 * s_full : 8 * n_sub]
                        )

            pending = None
            for o0, wks in kslices:
                hi = min(o0 + wks, K)
                dlen = hi - o0
                # one big x DMA + one m DMA per channel: 3 ring slots per
                # slice instead of 8, so DMA issue runs ~2.7 slices ahead of
                # the 8-deep per-ring in-flight window.
                x_t = xp.tile([128, 4 * maxw], F32, tag="x")
                nc.sync.dma_start(
                    out=x_t[:, :].rearrange("p (n k) -> p n k", n=4)[
                        :, :, 0:dlen
                    ],
                    in_=x.rearrange("(n p) k -> p n k", p=128)[:, :, o0:hi],
                )
                m_tc = []
                for c in range(C):
                    m_t = mp.tile([128, 4 * maxw], F32, tag="m")
                    m_eng.dma_start(
                        out=m_t[:, :].rearrange("p (n k) -> p n k", n=4)[
                            :, :, 0:dlen
                        ],
                        in_=mk[c].rearrange("(n p) k -> p n k", p=128)[
                            :, :, o0:hi
                        ],
                    )
                    m_tc.append(m_t)
                if o0 == kslices[0][0]:
                    load_w()
                mk_ts = {}
                for n in range(4):
                    for c in range(C):
                        mk_t = mkp.tile([128, maxw], MM_DT, tag="mk")
                        nc.vector.tensor_mul(
                            mk_t[:, 0:dlen],
                            x_t[:, n * maxw : n * maxw + dlen],
                            m_tc[c][:, n * maxw : n * maxw + dlen],
                        )
                        mk_ts[c, n] = mk_t

                # matmuls now (consume mk tiles promptly); tail one slice late
                pos = {c: [] for c in range(C)}
                for c in range(C):
                    q0 = o0
                    while q0 < o0 + wks:
                        wch = min(512, o0 + wks - q0)
                        wa = min(wch, K - q0)  # columns actually streamed
                        po = pop.tile([40, 512], F32, tag="po")
                        for n in range(4):
                            nc.tensor.matmul(
                                po[0:40, 0:wa],
                                w_t[:, SW * n : SW * (n + 1)],
                                mk_ts[c, n][:, q0 - o0 : q0 - o0 + wa],
                                start=(n == 0),
                                stop=(n == 3),
                            )
                        pos[c].append((q0, wch, wa, po))
                        q0 += wch

                if not lag:
                    emit_tail(o0, wks, pos)
                else:
                    if pending is not None:
                        emit_tail(*pending)
                    pending = (o0, wks, pos)
            if pending is not None:
                emit_tail(*pending)
    nc.finalize()
    return nc


_NC_CACHE = None


def _get_nc():
    global _NC_CACHE
    if _NC_CACHE is None:
        import os

        cfg = os.environ.get("KCONF", "")
        kw = {}
        if cfg == "mixed":
            kw = {"kslices": KSLICES_MIXED}
        elif cfg == "mixed_lag":
            kw = {"kslices": KSLICES_MIXED, "lag": 1}
        elif cfg == "lag":
            kw = {"lag": 1}
        elif cfg == "mscalar":
            kw = {"m_ring": "scalar"}
        _NC_CACHE = _build_nc(**kw)
    return _NC_CACHE


def run(inputs, est_mask, W, trace=False):
    """Returns (out [M, C, T_OUT] float32, exec_time_ns or None)."""
    inputs = np.ascontiguousarray(np.asarray(inputs, dtype=np.float32))
    est_mask = np.ascontiguousarray(np.asarray(est_mask, dtype=np.float32))
    W = np.ascontiguousarray(np.asarray(W, dtype=np.float32))
    assert inputs.shape == (M, N, K)
    assert est_mask.shape == (M, C, N, K)
    assert W.shape == (N, L)

    nc = _get_nc()
    in_maps = [
        {"x": inputs[m], "mask": est_mask[m], "w": W} for m in range(M)
    ]
    res = run_bass_kernel_spmd(nc, in_maps, list(range(M)), trace=trace)
    out = np.stack([res.results[m]["out"] for m in range(M)], axis=0)
    return out.astype(np.float32, copy=False), res.exec_time_ns


def kernel(inputs, est_mask, W):
    out, _ = run(inputs, est_mask, W)
    return out
